# revision 19
# baseline (speedup 1.0000x reference)
"""Trainium2 Bass kernel for nn_ExpansionFX (e3nn-style expansion tensor product).

Self-contained: hardcodes shapes/sharding. kernel(**inputs) takes FULL inputs,
shards batch across 8 NeuronCores, runs the Bass kernel, returns FULL output.

Math (per batch element b):
  for each instruction (i, j, k) [input irrep i -> (out1 irrep j, out2 irrep k)]:
    res[u,v,kk] = sum_w  weights[b, w,u,v] * x[b, w,kk]          (+ bias for i==0)
    out_block(j,k)[(u,ii),(v,jj)] += w3j[ii,jj,kk] * res[u,v,kk] / mul_in

Kernel mapping:
  Stage 1 (PE): per group of nb=128//mul batch elems, matmul with
    lhsT = weights arranged [(t,w) partitions, uv cols]  (stationary)
    rhs  = block-diag x bundle [(t,w) partitions, (k,t') cols]
    -> psum res_T [uv, (g,k,t)]  (feature-major)
  Evac+permute (ACT): psum -> SBUF res_T [uv, k*128 + g*nb + t]
  Transpose (PE): res_T[:, k-slice] -> batch-major res_bm [b, feature]
  Stage 2 (DVE): strided scalar_tensor_tensor FMA ops apply w3j coefficients
    (+ prescaled bias) writing the reference [60,60] layout directly.
"""

import math
import os

import numpy as np

# ---------------- problem constants (hardcoded) ----------------
IRREPS_IN = [(32, 0), (16, 1), (8, 2)]
IRREPS_O1 = [(16, 0), (8, 1), (4, 2)]
IRREPS_O2 = [(16, 0), (8, 1), (4, 2)]
BATCH = 8192
NCORES = 8
BS = BATCH // NCORES          # 1024 per core
TILE = 128
NT = int(os.environ.get("KT_NT", BS // TILE))  # batch tiles per core (8)
DIM_IN = 120
NUM_W = 19328
NUM_B = 336
OUT_D = 60
RO = [0, 16, 40]              # out1 block row offsets
CO = [0, 16, 40]              # out2 block col offsets
XBW = 128 + 384 + 640         # x-bundle width per tile
W_DT = os.environ.get("KT_DT", "f32")  # "f32" | "bf16" stage-1 dtype


# ---------------- wigner 3j (same math as reference) ----------------
def _three_j(j1, j2, j3, m1, m2, m3):
    if m1 + m2 + m3 != 0:
        return 0.0
    if j3 < abs(j1 - j2) or j3 > j1 + j2:
        return 0.0
    f = math.factorial
    tmin = max(0, j2 - j3 - m1, j1 - j3 + m2)
    tmax = min(j1 + j2 - j3, j1 - m1, j2 + m2)
    s = 0.0
    for t in range(tmin, tmax + 1):
        s += (-1) ** t / (f(t) * f(j3 - j2 + m1 + t) * f(j3 - j1 - m2 + t)
                          * f(j1 + j2 - j3 - t) * f(j1 - m1 - t) * f(j2 + m2 - t))
    delta = f(j1 + j2 - j3) * f(j1 - j2 + j3) * f(-j1 + j2 + j3) / f(j1 + j2 + j3 + 1)
    pref = math.sqrt(delta * f(j1 + m1) * f(j1 - m1) * f(j2 + m2) * f(j2 - m2)
                     * f(j3 + m3) * f(j3 - m3))
    return (-1) ** (j1 - j2 - m3) * pref * s


def _U_real(l):
    U = np.zeros((2 * l + 1, 2 * l + 1), dtype=complex)
    s = 1.0 / math.sqrt(2.0)
    U[l, l] = 1.0
    for m in range(1, l + 1):
        U[l + m, l - m] = s
        U[l + m, l + m] = s * (-1) ** m
        U[l - m, l - m] = 1j * s
        U[l - m, l + m] = -1j * s * (-1) ** m
    return U


def _real_w3j(l1, l2, l3):
    T = np.zeros((2 * l1 + 1, 2 * l2 + 1, 2 * l3 + 1), dtype=complex)
    for m1 in range(-l1, l1 + 1):
        for m2 in range(-l2, l2 + 1):
            m3 = -m1 - m2
            if -l3 <= m3 <= l3:
                T[l1 + m1, l2 + m2, l3 + m3] = _three_j(l1, l2, l3, m1, m2, m3)
    Tr = np.einsum('ai,bj,ck,ijk->abc', _U_real(l1), _U_real(l2), _U_real(l3), T)
    re, im = Tr.real, Tr.imag
    M = re if np.linalg.norm(re) >= np.linalg.norm(im) else im
    n = np.linalg.norm(M)
    return (M / n if n > 0 else M).astype(np.float32)


# ---------------- instruction / layout metadata ----------------
def _ref_instructions():
    ins = []
    for i, (ni, li) in enumerate(IRREPS_IN):
        for j, (n1, l1) in enumerate(IRREPS_O1):
            for k, (n2, l2) in enumerate(IRREPS_O2):
                if abs(l1 - l2) <= li <= l1 + l2:
                    ins.append((i, j, k))
    return ins


REF_INS = _ref_instructions()
# reference per-instruction weight offsets
_REF_WOFF = {}
_o = 0
for (i, j, k) in REF_INS:
    _REF_WOFF[(i, j, k)] = _o
    _o += IRREPS_IN[i][0] * IRREPS_O1[j][0] * IRREPS_O2[k][0]
assert _o == NUM_W
# reference w3j flat offsets (j_off order) and bias offsets
_REF_JOFF = {}
_o = 0
for (i, j, k) in REF_INS:
    d1 = 2 * IRREPS_O1[j][1] + 1
    d2 = 2 * IRREPS_O2[k][1] + 1
    din = 2 * IRREPS_IN[i][1] + 1
    _REF_JOFF[(i, j, k)] = _o
    _o += d1 * d2 * din
_REF_BOFF = {(0, 0, 0): 0, (0, 1, 1): 256, (0, 2, 2): 320}

# region instruction order (i2 reordered so chunk sub-bases are 32-aligned legal)
_REGION_ORDER = [
    [(0, 0), (1, 1), (2, 2)],
    [(0, 1), (1, 0), (1, 1), (1, 2), (2, 1), (2, 2)],
    [(0, 2), (1, 1), (2, 0), (1, 2), (2, 1), (2, 2)],
]
_CHUNKS = [
    [(0, 128), (128, 128), (256, 80)],
    [(0, 128), (128, 128), (256, 128), (384, 16)],
    [(0, 128), (128, 128), (256, 16)],
]


def _build_regions():
    regions = []
    colR = 0
    bmoff = 0
    xoff = 0
    xbase = 0
    for r, (mul, li) in enumerate(IRREPS_IN):
        nb = 128 // mul
        din = 2 * li + 1
        G = TILE // nb
        instrs = []
        uvc = 0
        for (j, k) in _REGION_ORDER[r]:
            n1 = IRREPS_O1[j][0]
            n2 = IRREPS_O2[k][0]
            instrs.append(dict(j=j, k=k, n1=n1, n2=n2, uvc=uvc,
                               woff=_REF_WOFF[(r, j, k)]))
            uvc += n1 * n2
        regW = uvc
        regions.append(dict(r=r, mul=mul, li=li, nb=nb, din=din, G=G,
                            regW=regW, colR=colR, bmoff=bmoff, xoff=xoff,
                            xbase=xbase, instrs=instrs, chunks=_CHUNKS[r]))
        colR += G * regW
        bmoff += regW * din
        xoff += mul * din
        xbase += TILE * din
    assert colR == NUM_W and xoff == DIM_IN and xbase == XBW
    return regions


REGIONS = _build_regions()
RES_BM_W = sum(rg["regW"] * rg["din"] for rg in REGIONS)  # 2896


# ---------------- stage-2 op list (with affine-chain merging) ----------------
def _build_ops(w3j_flat):
    """Each op: dict(kind='fma'|'bias', coef, src(base, chain_step),
    dst(base, chain_step), chain_len, n1, n2, d1, d2, bias_base or None)."""
    ops = []
    for rg in REGIONS:
        r = rg["r"]
        mul = rg["mul"]
        regW = rg["regW"]
        bm = rg["bmoff"]
        for ins in rg["instrs"]:
            j, k = ins["j"], ins["k"]
            n1, n2 = ins["n1"], ins["n2"]
            d1 = 2 * IRREPS_O1[j][1] + 1
            d2 = 2 * IRREPS_O2[k][1] + 1
            din = rg["din"]
            jo = _REF_JOFF[(r, j, k)]
            W = np.asarray(w3j_flat[jo:jo + d1 * d2 * din]).reshape(d1, d2, din)
            nz = [(ii, jj, kk, float(W[ii, jj, kk]) / mul)
                  for ii in range(d1) for jj in range(d2) for kk in range(din)
                  if abs(W[ii, jj, kk]) > 1e-9]
            has_bias = (r == 0)
            # group by coefficient value, merge arithmetic chains
            nz_sorted = sorted(nz, key=lambda t: (round(t[3], 9), t[0], t[1], t[2]))
            groups = []
            for t in nz_sorted:
                if groups and abs(groups[-1][0] - t[3]) < 1e-9:
                    groups[-1][1].append(t[:3])
                else:
                    groups.append((t[3], [t[:3]]))
            for coef, trips in groups:
                # ScalarTensorTensor is limited to 2 free dims on TRN2's
                # walrus, so no chain merging: one op per nonzero.
                for (i0, j0, k0) in trips:
                    src_base = bm + k0 * regW + ins["uvc"]
                    dst_base = (RO[j] + 0 * d1 + i0) * OUT_D + CO[k] + j0
                    ops.append(dict(
                        coef=coef, chain=1, r=r, kk=k0,
                        src_base=ins["uvc"],
                        dst_base=dst_base, dst_cs=0,
                        n1=n1, n2=n2, d1=d1, d2=d2,
                        u_dst=d1 * OUT_D, v_dst=d2, u_src=n2,
                        bias_base=(_REF_BOFF[(0, j, k)] if has_bias else None),
                    ))
    # sort by (region, k) so each op runs as soon as its res slice is
    # transposed; region 0 (bias writers) naturally comes first
    ops.sort(key=lambda o: (o["r"], o["kk"],
                            0 if o["bias_base"] is not None else 1))
    return ops


# ---------------- host-side data prep ----------------
def _prep_core(x, w, bw, w3j_list, np_dt):
    """x [BS,120], w [BS,19328], bw [BS,336] -> wre, xb, biasp arrays."""
    nt = BS // TILE
    Wt = np.ascontiguousarray(w).reshape(nt, TILE, NUM_W)
    wre = np.empty((nt, TILE, NUM_W), dtype=np.float32)
    xb = np.zeros((nt, TILE, XBW), dtype=np.float32)
    for rg in REGIONS:
        mul, nb, din, G, regW = rg["mul"], rg["nb"], rg["din"], rg["G"], rg["regW"]
        dst = wre[:, :, rg["colR"]:rg["colR"] + G * regW].reshape(nt, TILE, G, regW)
        for ins in rg["instrs"]:
            n12 = ins["n1"] * ins["n2"]
            src = Wt[:, :, ins["woff"]:ins["woff"] + mul * n12]
            src = src.reshape(nt, G, nb, mul, n12)
            # dst[p = t*mul + w, g, uvc + uv] = src[g, t, w, uv]
            dst[:, :, :, ins["uvc"]:ins["uvc"] + n12] = (
                src.transpose(0, 2, 3, 1, 4).reshape(nt, TILE, G, n12))
        xs = np.ascontiguousarray(x[:, rg["xoff"]:rg["xoff"] + mul * din])
        xs = xs.reshape(nt, G, nb, mul, din)
        xv = xb[:, :, rg["xbase"]:rg["xbase"] + TILE * din]
        xv = xv.reshape(nt, nb, mul, G, din, nb)
        for t in range(nb):
            # partition block t, col (g, k, t'=t)
            xv[:, t, :, :, :, t] = xs[:, :, t].transpose(0, 2, 1, 3)
    biasp = np.ascontiguousarray(bw).reshape(nt, TILE, NUM_B).astype(np.float32)
    scal = np.empty(NUM_B, np.float32)
    for (jj, (j, k)) in enumerate(_REGION_ORDER[0]):
        jo = _REF_JOFF[(0, j, k)]
        d1 = 2 * IRREPS_O1[j][1] + 1
        val = float(np.asarray(w3j_list[jo]))  # diagonal value (first elem)
        bo = _REF_BOFF[(0, j, k)]
        n12 = IRREPS_O1[j][0] * IRREPS_O2[k][0]
        scal[bo:bo + n12] = val / 32.0
    biasp *= scal[None, None, :]
    return wre.astype(np_dt, copy=False), xb.astype(np_dt, copy=False), biasp


# ---------------- bass program ----------------
_PROG = {}


def _build_program(w3j_flat):
    from contextlib import ExitStack
    import concourse.bass as bass
    import concourse.tile as tile
    from concourse import bacc, mybir
    from concourse.masks import make_identity

    f32 = mybir.dt.float32
    dt_w = f32 if W_DT == "f32" else mybir.dt.bfloat16

    nc = bacc.Bacc("TRN2", target_bir_lowering=False, debug=False,
                   num_devices=NCORES)
    wre_d = nc.dram_tensor("wre", [NT, TILE, NUM_W], dt_w, kind="ExternalInput")
    xb_d = nc.dram_tensor("xb", [NT, TILE, XBW], dt_w, kind="ExternalInput")
    bi_d = nc.dram_tensor("biasp", [NT, TILE, NUM_B], f32, kind="ExternalInput")
    out_d = nc.dram_tensor("out", [NT, TILE, OUT_D * OUT_D], f32,
                           kind="ExternalOutput")

    ops = _build_ops(w3j_flat)

    # weight DMA sub-tiles: (region, g0, ngroups)
    wsplits = [(0, g, 8) for g in range(0, 32, 8)] + \
              [(1, g, 8) for g in range(0, 16, 8)] + \
              [(2, 0, 8)]

    with tile.TileContext(nc) as tc, ExitStack() as ctx:
        def pool(name, bufs, space=None):
            kw = dict(name=name, bufs=bufs)
            if space:
                kw["space"] = space
            return ctx.enter_context(tc.tile_pool(**kw))

        import os as _os
        _bufs = _os.environ.get("KT_BUFS", "2,2,2,2,2,4,2,2,4,3").split(",")
        b_w0, b_w1, b_w2, b_xp, b_bp, b_rtp, b_rbp, b_outp, b_ps1, b_pst = \
            [int(x) for x in _bufs]
        wpools = {0: pool("w0", b_w0), 1: pool("w1", b_w1), 2: pool("w2", b_w2)}
        xp = pool("xbp", b_xp)
        bp = pool("biasb", b_bp)
        rtp = pool("resT", b_rtp)
        rbp = pool("resbm", b_rbp)
        outp = pool("outp", b_outp)
        idp = pool("ident", 1)
        ps1 = pool("ps1", b_ps1, space="PSUM")
        pst = pool("pstr", b_pst, space="PSUM")

        ident = idp.tile([128, 128], f32)
        make_identity(nc, ident[:])

        for t in range(NT):
            # ---- loads ----
            wtiles = {}
            for (r, g0, ng) in wsplits:
                rg = REGIONS[r]
                cols = ng * rg["regW"]
                wt = wpools[r].tile([TILE, cols], dt_w, tag=f"w{r}")
                c0 = rg["colR"] + g0 * rg["regW"]
                nc.sync.dma_start(wt[:], wre_d[t, :, c0:c0 + cols])
                wtiles[(r, g0)] = wt
            xbt = xp.tile([TILE, XBW], dt_w)
            nc.sync.dma_start(xbt[:], xb_d[t])
            bit = bp.tile([TILE, NUM_B], f32)
            nc.sync.dma_start(bit[:], bi_d[t])

            bands = [(0, 16), (16, 40), (40, 60)]
            outts = []
            for (r0, r1) in bands:
                ot = outp.tile([TILE, r1 - r0, OUT_D], f32, tag=f"out{r0}")
                nc.gpsimd.memset(ot[:], 0.0)
                outts.append(ot)
            res_k = {}
            for rg in REGIONS:
                for kk in range(rg["din"]):
                    res_k[(rg["r"], kk)] = rbp.tile(
                        [TILE, rg["regW"]], f32, tag=f"rb{rg['r']}_{kk}",
                        name=f"rk{t}_{rg['r']}_{kk}")

            # ---- stage 1 + transpose per region ----
            for rg in REGIONS:
                r, nb, din, G, regW = rg["r"], rg["nb"], rg["din"], rg["G"], rg["regW"]
                nbd = nb * din
                # psum column capacity: groups per psum tile
                gh = G if G * nbd <= 512 else G // 2
                for (c0, rows) in rg["chunks"]:
                    resT = rtp.tile([rows, din * TILE], f32, tag="resT")
                    for h0 in range(0, G, gh):
                        pt = ps1.tile([rows, gh * nbd], f32, tag="ps1")
                        for g in range(h0, h0 + gh):
                            wt = wtiles[(r, (g // 8) * 8)]
                            gl = g % 8
                            lhsT = wt[:, gl * regW + c0: gl * regW + c0 + rows]
                            rhs = xbt[:, rg["xbase"] + g * nbd:
                                      rg["xbase"] + (g + 1) * nbd]
                            mo = (g - h0) * nbd
                            nc.tensor.matmul(pt[:, mo:mo + nbd], lhsT, rhs,
                                             start=True, stop=True)
                        # evac + permute (g,k,t) -> col k*128 + g*nb + t
                        src = pt[:].rearrange("p (g k t) -> p g k t", k=din, t=nb)
                        dst = resT[:].rearrange("p (k g t) -> p g k t", k=din, t=nb)
                        nc.scalar.copy(out=dst[:, h0:h0 + gh], in_=src)
                    for kk in range(din):
                        ptr = pst.tile([TILE, rows], f32, tag="pstr")
                        nc.tensor.transpose(ptr[:], resT[:, kk * TILE:(kk + 1) * TILE],
                                            ident[0:rows, 0:rows])
                        nc.scalar.copy(out=res_k[(r, kk)][:, c0:c0 + rows],
                                       in_=ptr[:])

            # ---- stage 2 (DVE) ----
            pdim_b = list(bit[:].ap[0])
            band_of = lambda o: 0 if o["dst_base"] // OUT_D < 16 else \
                (1 if o["dst_base"] // OUT_D < 40 else 2)
            # each band's out-DMA fires after its last op (in sorted order)
            last_in_band = {}
            for oi, op in enumerate(ops):
                last_in_band[band_of(op)] = oi
            for oi, op in enumerate(ops):
                n1, n2 = op["n1"], op["n2"]
                b = band_of(op)
                ot = outts[b]
                rb = res_k[(op["r"], op["kk"])]
                src = bass.AP(rb[:].tensor, rb[:].offset + op["src_base"],
                              [list(rb[:].ap[0]), [op["u_src"], n1], [1, n2]])
                dst = bass.AP(ot[:].tensor,
                              ot[:].offset + op["dst_base"] - bands[b][0] * OUT_D,
                              [list(ot[:].ap[0]), [op["u_dst"], n1],
                               [op["v_dst"], n2]])
                if op["bias_base"] is not None:
                    other = bass.AP(bit[:].tensor, bit[:].offset + op["bias_base"],
                                    [pdim_b, [n2, n1], [1, n2]])
                else:
                    other = dst
                nc.vector.scalar_tensor_tensor(
                    out=dst, in0=src, scalar=float(op["coef"]), in1=other,
                    op0=mybir.AluOpType.mult, op1=mybir.AluOpType.add)
                if last_in_band[b] == oi:
                    r0, r1 = bands[b]
                    nc.gpsimd.dma_start(
                        out_d[t, :, r0 * OUT_D:r1 * OUT_D],
                        ot[:].rearrange("p a b -> p (a b)"))

    nc.compile()
    return nc


def _get_prog(w3j_flat):
    key = (W_DT, NT)
    if key not in _PROG:
        _PROG[key] = _build_program(w3j_flat)
    return _PROG[key]


# ---------------- entry point ----------------
def kernel(x_in, weights, bias_weights, w3j_flat, _trace=False):
    import ml_dtypes
    from concourse.bass_utils import run_bass_kernel_spmd

    x_in = np.asarray(x_in, dtype=np.float32)
    weights = np.asarray(weights, dtype=np.float32)
    bias_weights = np.asarray(bias_weights, dtype=np.float32)
    w3j_flat = np.asarray(w3j_flat, dtype=np.float32)

    nc = _get_prog(w3j_flat)
    np_dt = np.float32 if W_DT == "f32" else ml_dtypes.bfloat16

    rows = NT * TILE  # rows actually computed per core
    in_maps = []
    for c in range(NCORES):
        sl = slice(c * BS, c * BS + BS)
        wre, xb, biasp = _prep_core(x_in[sl], weights[sl], bias_weights[sl],
                                    w3j_flat, np_dt)
        in_maps.append(dict(wre=wre[:NT], xb=xb[:NT], biasp=biasp[:NT]))

    res = run_bass_kernel_spmd(nc, in_maps, core_ids=list(range(NCORES)),
                               trace=_trace)
    outs = [r["out"].reshape(rows, OUT_D, OUT_D) for r in res.results]
    if NT == BS // TILE:
        full = np.concatenate(outs, axis=0)
    else:  # debug partial build
        full = np.zeros((BATCH, OUT_D, OUT_D), np.float32)
        for c in range(NCORES):
            full[c * BS:c * BS + rows] = outs[c]
    kernel.last_exec_ns = res.exec_time_ns
    return full


kernel.last_exec_ns = None


# revision 20
# speedup vs baseline: 1.1327x; 1.1327x over previous
"""Trainium2 Bass kernel for nn_ExpansionFX (e3nn-style expansion tensor product).

Self-contained: hardcodes shapes/sharding. kernel(**inputs) takes FULL inputs,
shards batch across 8 NeuronCores, runs the Bass kernel, returns FULL output.

Math (per batch element b):
  for each instruction (i, j, k) [input irrep i -> (out1 irrep j, out2 irrep k)]:
    res[u,v,kk] = sum_w  weights[b, w,u,v] * x[b, w,kk]          (+ bias for i==0)
    out_block(j,k)[(u,ii),(v,jj)] += w3j[ii,jj,kk] * res[u,v,kk] / mul_in

Kernel mapping:
  Stage 1 (PE): per group of nb=128//mul batch elems, matmul with
    lhsT = weights arranged [(t,w) partitions, uv cols]  (stationary)
    rhs  = block-diag x bundle [(t,w) partitions, (k,t') cols]
    -> psum res_T [uv, (g,k,t)]  (feature-major)
  Evac+permute (ACT): psum -> SBUF res_T [uv, k*128 + g*nb + t]
  Transpose (PE): res_T[:, k-slice] -> batch-major res_bm [b, feature]
  Stage 2 (DVE): strided scalar_tensor_tensor FMA ops apply w3j coefficients
    (+ prescaled bias) writing the reference [60,60] layout directly.
"""

import math
import os

import numpy as np

# ---------------- problem constants (hardcoded) ----------------
IRREPS_IN = [(32, 0), (16, 1), (8, 2)]
IRREPS_O1 = [(16, 0), (8, 1), (4, 2)]
IRREPS_O2 = [(16, 0), (8, 1), (4, 2)]
BATCH = 8192
NCORES = 8
BS = BATCH // NCORES          # 1024 per core
TILE = 128
NT = int(os.environ.get("KT_NT", BS // TILE))  # batch tiles per core (8)
DIM_IN = 120
NUM_W = 19328
NUM_B = 336
OUT_D = 60
RO = [0, 16, 40]              # out1 block row offsets
CO = [0, 16, 40]              # out2 block col offsets
XBW = 128 + 384 + 640         # x-bundle width per tile
W_DT = os.environ.get("KT_DT", "f32")  # "f32" | "bf16" stage-1 dtype


# ---------------- wigner 3j (same math as reference) ----------------
def _three_j(j1, j2, j3, m1, m2, m3):
    if m1 + m2 + m3 != 0:
        return 0.0
    if j3 < abs(j1 - j2) or j3 > j1 + j2:
        return 0.0
    f = math.factorial
    tmin = max(0, j2 - j3 - m1, j1 - j3 + m2)
    tmax = min(j1 + j2 - j3, j1 - m1, j2 + m2)
    s = 0.0
    for t in range(tmin, tmax + 1):
        s += (-1) ** t / (f(t) * f(j3 - j2 + m1 + t) * f(j3 - j1 - m2 + t)
                          * f(j1 + j2 - j3 - t) * f(j1 - m1 - t) * f(j2 + m2 - t))
    delta = f(j1 + j2 - j3) * f(j1 - j2 + j3) * f(-j1 + j2 + j3) / f(j1 + j2 + j3 + 1)
    pref = math.sqrt(delta * f(j1 + m1) * f(j1 - m1) * f(j2 + m2) * f(j2 - m2)
                     * f(j3 + m3) * f(j3 - m3))
    return (-1) ** (j1 - j2 - m3) * pref * s


def _U_real(l):
    U = np.zeros((2 * l + 1, 2 * l + 1), dtype=complex)
    s = 1.0 / math.sqrt(2.0)
    U[l, l] = 1.0
    for m in range(1, l + 1):
        U[l + m, l - m] = s
        U[l + m, l + m] = s * (-1) ** m
        U[l - m, l - m] = 1j * s
        U[l - m, l + m] = -1j * s * (-1) ** m
    return U


def _real_w3j(l1, l2, l3):
    T = np.zeros((2 * l1 + 1, 2 * l2 + 1, 2 * l3 + 1), dtype=complex)
    for m1 in range(-l1, l1 + 1):
        for m2 in range(-l2, l2 + 1):
            m3 = -m1 - m2
            if -l3 <= m3 <= l3:
                T[l1 + m1, l2 + m2, l3 + m3] = _three_j(l1, l2, l3, m1, m2, m3)
    Tr = np.einsum('ai,bj,ck,ijk->abc', _U_real(l1), _U_real(l2), _U_real(l3), T)
    re, im = Tr.real, Tr.imag
    M = re if np.linalg.norm(re) >= np.linalg.norm(im) else im
    n = np.linalg.norm(M)
    return (M / n if n > 0 else M).astype(np.float32)


# ---------------- instruction / layout metadata ----------------
def _ref_instructions():
    ins = []
    for i, (ni, li) in enumerate(IRREPS_IN):
        for j, (n1, l1) in enumerate(IRREPS_O1):
            for k, (n2, l2) in enumerate(IRREPS_O2):
                if abs(l1 - l2) <= li <= l1 + l2:
                    ins.append((i, j, k))
    return ins


REF_INS = _ref_instructions()
# reference per-instruction weight offsets
_REF_WOFF = {}
_o = 0
for (i, j, k) in REF_INS:
    _REF_WOFF[(i, j, k)] = _o
    _o += IRREPS_IN[i][0] * IRREPS_O1[j][0] * IRREPS_O2[k][0]
assert _o == NUM_W
# reference w3j flat offsets (j_off order) and bias offsets
_REF_JOFF = {}
_o = 0
for (i, j, k) in REF_INS:
    d1 = 2 * IRREPS_O1[j][1] + 1
    d2 = 2 * IRREPS_O2[k][1] + 1
    din = 2 * IRREPS_IN[i][1] + 1
    _REF_JOFF[(i, j, k)] = _o
    _o += d1 * d2 * din
_REF_BOFF = {(0, 0, 0): 0, (0, 1, 1): 256, (0, 2, 2): 320}

# region instruction order (i2 reordered so chunk sub-bases are 32-aligned legal)
_REGION_ORDER = [
    [(0, 0), (1, 1), (2, 2)],
    [(0, 1), (1, 0), (1, 1), (1, 2), (2, 1), (2, 2)],
    [(0, 2), (1, 1), (2, 0), (1, 2), (2, 1), (2, 2)],
]
_CHUNKS = [
    [(0, 128), (128, 128), (256, 80)],
    [(0, 128), (128, 128), (256, 128), (384, 16)],
    [(0, 128), (128, 128), (256, 16)],
]


def _build_regions():
    regions = []
    colR = 0
    bmoff = 0
    xoff = 0
    xbase = 0
    for r, (mul, li) in enumerate(IRREPS_IN):
        nb = 128 // mul
        din = 2 * li + 1
        G = TILE // nb
        instrs = []
        uvc = 0
        for (j, k) in _REGION_ORDER[r]:
            n1 = IRREPS_O1[j][0]
            n2 = IRREPS_O2[k][0]
            instrs.append(dict(j=j, k=k, n1=n1, n2=n2, uvc=uvc,
                               woff=_REF_WOFF[(r, j, k)]))
            uvc += n1 * n2
        regW = uvc
        regions.append(dict(r=r, mul=mul, li=li, nb=nb, din=din, G=G,
                            regW=regW, colR=colR, bmoff=bmoff, xoff=xoff,
                            xbase=xbase, instrs=instrs, chunks=_CHUNKS[r]))
        colR += G * regW
        bmoff += regW * din
        xoff += mul * din
        xbase += TILE * din
    assert colR == NUM_W and xoff == DIM_IN and xbase == XBW
    return regions


REGIONS = _build_regions()
RES_BM_W = sum(rg["regW"] * rg["din"] for rg in REGIONS)  # 2896


# ---------------- stage-2 op list (with affine-chain merging) ----------------
def _build_ops(w3j_flat):
    """Each op: dict(kind='fma'|'bias', coef, src(base, chain_step),
    dst(base, chain_step), chain_len, n1, n2, d1, d2, bias_base or None)."""
    ops = []
    for rg in REGIONS:
        r = rg["r"]
        mul = rg["mul"]
        regW = rg["regW"]
        bm = rg["bmoff"]
        for ins in rg["instrs"]:
            j, k = ins["j"], ins["k"]
            n1, n2 = ins["n1"], ins["n2"]
            d1 = 2 * IRREPS_O1[j][1] + 1
            d2 = 2 * IRREPS_O2[k][1] + 1
            din = rg["din"]
            jo = _REF_JOFF[(r, j, k)]
            W = np.asarray(w3j_flat[jo:jo + d1 * d2 * din]).reshape(d1, d2, din)
            nz = [(ii, jj, kk, float(W[ii, jj, kk]) / mul)
                  for ii in range(d1) for jj in range(d2) for kk in range(din)
                  if abs(W[ii, jj, kk]) > 1e-9]
            has_bias = (r == 0)
            # group by coefficient value, merge arithmetic chains
            nz_sorted = sorted(nz, key=lambda t: (round(t[3], 9), t[0], t[1], t[2]))
            groups = []
            for t in nz_sorted:
                if groups and abs(groups[-1][0] - t[3]) < 1e-9:
                    groups[-1][1].append(t[:3])
                else:
                    groups.append((t[3], [t[:3]]))
            for coef, trips in groups:
                # ScalarTensorTensor is limited to 2 free dims on TRN2's
                # walrus, so no chain merging: one op per nonzero.
                for (i0, j0, k0) in trips:
                    src_base = bm + k0 * regW + ins["uvc"]
                    dst_base = (RO[j] + 0 * d1 + i0) * OUT_D + CO[k] + j0
                    ops.append(dict(
                        coef=coef, chain=1, r=r, kk=k0,
                        src_base=ins["uvc"],
                        dst_base=dst_base, dst_cs=0,
                        n1=n1, n2=n2, d1=d1, d2=d2,
                        u_dst=d1 * OUT_D, v_dst=d2, u_src=n2,
                        bias_base=(_REF_BOFF[(0, j, k)] if has_bias else None),
                    ))
    # sort by (region, k) so each op runs as soon as its res slice is
    # transposed; region 0 (bias writers) naturally comes first
    ops.sort(key=lambda o: (o["r"], o["kk"],
                            0 if o["bias_base"] is not None else 1))
    return ops


# ---------------- host-side data prep ----------------
def _prep_core(x, w, bw, w3j_list, np_dt):
    """x [BS,120], w [BS,19328], bw [BS,336] -> wre, xb, biasp arrays."""
    nt = BS // TILE
    Wt = np.ascontiguousarray(w).reshape(nt, TILE, NUM_W)
    wre = np.empty((nt, TILE, NUM_W), dtype=np.float32)
    xb = np.zeros((nt, TILE, XBW), dtype=np.float32)
    for rg in REGIONS:
        mul, nb, din, G, regW = rg["mul"], rg["nb"], rg["din"], rg["G"], rg["regW"]
        dst = wre[:, :, rg["colR"]:rg["colR"] + G * regW].reshape(nt, TILE, G, regW)
        for ins in rg["instrs"]:
            n12 = ins["n1"] * ins["n2"]
            src = Wt[:, :, ins["woff"]:ins["woff"] + mul * n12]
            src = src.reshape(nt, G, nb, mul, n12)
            # dst[p = t*mul + w, g, uvc + uv] = src[g, t, w, uv]
            dst[:, :, :, ins["uvc"]:ins["uvc"] + n12] = (
                src.transpose(0, 2, 3, 1, 4).reshape(nt, TILE, G, n12))
        xs = np.ascontiguousarray(x[:, rg["xoff"]:rg["xoff"] + mul * din])
        xs = xs.reshape(nt, G, nb, mul, din)
        xv = xb[:, :, rg["xbase"]:rg["xbase"] + TILE * din]
        xv = xv.reshape(nt, nb, mul, G, din, nb)
        for t in range(nb):
            # partition block t, col (g, k, t'=t)
            xv[:, t, :, :, :, t] = xs[:, :, t].transpose(0, 2, 1, 3)
    biasp = np.ascontiguousarray(bw).reshape(nt, TILE, NUM_B).astype(np.float32)
    scal = np.empty(NUM_B, np.float32)
    for (jj, (j, k)) in enumerate(_REGION_ORDER[0]):
        jo = _REF_JOFF[(0, j, k)]
        d1 = 2 * IRREPS_O1[j][1] + 1
        val = float(np.asarray(w3j_list[jo]))  # diagonal value (first elem)
        bo = _REF_BOFF[(0, j, k)]
        n12 = IRREPS_O1[j][0] * IRREPS_O2[k][0]
        scal[bo:bo + n12] = val / 32.0
    biasp *= scal[None, None, :]
    return wre.astype(np_dt, copy=False), xb.astype(np_dt, copy=False), biasp


# ---------------- bass program ----------------
_PROG = {}


def _build_program(w3j_flat):
    from contextlib import ExitStack
    import concourse.bass as bass
    import concourse.tile as tile
    from concourse import bacc, mybir
    from concourse.masks import make_identity

    f32 = mybir.dt.float32
    dt_w = f32 if W_DT == "f32" else mybir.dt.bfloat16

    nc = bacc.Bacc("TRN2", target_bir_lowering=False, debug=False,
                   num_devices=NCORES)
    wre_d = nc.dram_tensor("wre", [NT, TILE, NUM_W], dt_w, kind="ExternalInput")
    xb_d = nc.dram_tensor("xb", [NT, TILE, XBW], dt_w, kind="ExternalInput")
    bi_d = nc.dram_tensor("biasp", [NT, TILE, NUM_B], f32, kind="ExternalInput")
    out_d = nc.dram_tensor("out", [NT, TILE, OUT_D * OUT_D], f32,
                           kind="ExternalOutput")

    ops = _build_ops(w3j_flat)

    # weight DMA sub-tiles: (region, g0, ngroups)
    wsplits = [(0, g, 8) for g in range(0, 32, 8)] + \
              [(1, g, 8) for g in range(0, 16, 8)] + \
              [(2, 0, 8)]

    with tile.TileContext(nc) as tc, ExitStack() as ctx:
        def pool(name, bufs, space=None):
            kw = dict(name=name, bufs=bufs)
            if space:
                kw["space"] = space
            return ctx.enter_context(tc.tile_pool(**kw))

        import os as _os
        _bufs = _os.environ.get("KT_BUFS", "2,2,2,2,2,4,2,2,4,3").split(",")
        b_w0, b_w1, b_w2, b_xp, b_bp, b_rtp, b_rbp, b_outp, b_ps1, b_pst = \
            [int(x) for x in _bufs]
        wpools = {0: pool("w0", b_w0), 1: pool("w1", b_w1), 2: pool("w2", b_w2)}
        xp = pool("xbp", b_xp)
        bp = pool("biasb", b_bp)
        rtp = pool("resT", b_rtp)
        rbp = pool("resbm", b_rbp)
        outp = pool("outp", b_outp)
        idp = pool("ident", 1)
        ps1 = pool("ps1", b_ps1, space="PSUM")
        pst = pool("pstr", b_pst, space="PSUM")

        ident = idp.tile([128, 128], f32)
        make_identity(nc, ident[:])

        for t in range(NT):
            # ---- loads ----
            wtiles = {}
            for (r, g0, ng) in wsplits:
                rg = REGIONS[r]
                cols = ng * rg["regW"]
                wt = wpools[r].tile([TILE, cols], dt_w, tag=f"w{r}")
                c0 = rg["colR"] + g0 * rg["regW"]
                nc.sync.dma_start(wt[:], wre_d[t, :, c0:c0 + cols])
                wtiles[(r, g0)] = wt
            xbt = xp.tile([TILE, XBW], dt_w)
            nc.sync.dma_start(xbt[:], xb_d[t])
            bit = bp.tile([TILE, NUM_B], f32)
            nc.sync.dma_start(bit[:], bi_d[t])

            bands = [(0, 16), (16, 40), (40, 60)]
            outts = []
            for (r0, r1) in bands:
                ot = outp.tile([TILE, r1 - r0, OUT_D], f32, tag=f"out{r0}")
                nc.gpsimd.memset(ot[:], 0.0)
                outts.append(ot)
            res_k = {}
            for rg in REGIONS:
                for kk in range(rg["din"]):
                    res_k[(rg["r"], kk)] = rbp.tile(
                        [TILE, rg["regW"]], f32, tag=f"rb{rg['r']}_{kk}",
                        name=f"rk{t}_{rg['r']}_{kk}")

            # ---- stage 1 + transpose per region ----
            for rg in REGIONS:
                r, nb, din, G, regW = rg["r"], rg["nb"], rg["din"], rg["G"], rg["regW"]
                nbd = nb * din
                # psum column capacity: groups per psum tile
                gh = G if G * nbd <= 512 else G // 2
                for (c0, rows) in rg["chunks"]:
                    resT = rtp.tile([rows, din * TILE], f32, tag="resT")
                    for h0 in range(0, G, gh):
                        pt = ps1.tile([rows, gh * nbd], f32, tag="ps1")
                        for g in range(h0, h0 + gh):
                            wt = wtiles[(r, (g // 8) * 8)]
                            gl = g % 8
                            lhsT = wt[:, gl * regW + c0: gl * regW + c0 + rows]
                            rhs = xbt[:, rg["xbase"] + g * nbd:
                                      rg["xbase"] + (g + 1) * nbd]
                            mo = (g - h0) * nbd
                            nc.tensor.matmul(pt[:, mo:mo + nbd], lhsT, rhs,
                                             start=True, stop=True)
                        # evac + permute (g,k,t) -> col k*128 + g*nb + t
                        src = pt[:].rearrange("p (g k t) -> p g k t", k=din, t=nb)
                        dst = resT[:].rearrange("p (k g t) -> p g k t", k=din, t=nb)
                        nc.scalar.copy(out=dst[:, h0:h0 + gh], in_=src)
                    for kk in range(din):
                        ptr = pst.tile([TILE, rows], f32, tag="pstr")
                        nc.tensor.transpose(ptr[:], resT[:, kk * TILE:(kk + 1) * TILE],
                                            ident[0:rows, 0:rows])
                        nc.scalar.copy(out=res_k[(r, kk)][:, c0:c0 + rows],
                                       in_=ptr[:])

            # ---- stage 2 (DVE) ----
            pdim_b = list(bit[:].ap[0])
            band_of = lambda o: 0 if o["dst_base"] // OUT_D < 16 else \
                (1 if o["dst_base"] // OUT_D < 40 else 2)
            # each band's out-DMA fires after its last op (in sorted order)
            last_in_band = {}
            for oi, op in enumerate(ops):
                last_in_band[band_of(op)] = oi
            for oi, op in enumerate(ops):
                n1, n2 = op["n1"], op["n2"]
                b = band_of(op)
                ot = outts[b]
                rb = res_k[(op["r"], op["kk"])]
                src = bass.AP(rb[:].tensor, rb[:].offset + op["src_base"],
                              [list(rb[:].ap[0]), [op["u_src"], n1], [1, n2]])
                dst = bass.AP(ot[:].tensor,
                              ot[:].offset + op["dst_base"] - bands[b][0] * OUT_D,
                              [list(ot[:].ap[0]), [op["u_dst"], n1],
                               [op["v_dst"], n2]])
                if op["bias_base"] is not None:
                    other = bass.AP(bit[:].tensor, bit[:].offset + op["bias_base"],
                                    [pdim_b, [n2, n1], [1, n2]])
                else:
                    other = dst
                nc.vector.scalar_tensor_tensor(
                    out=dst, in0=src, scalar=float(op["coef"]), in1=other,
                    op0=mybir.AluOpType.mult, op1=mybir.AluOpType.add)
                if last_in_band[b] == oi:
                    r0, r1 = bands[b]
                    _eng = {"sync": nc.sync, "gpsimd": nc.gpsimd,
                            "scalar": nc.scalar, "vector": nc.vector}[
                        os.environ.get("KT_ODMA", "scalar")]
                    _eng.dma_start(
                        out_d[t, :, r0 * OUT_D:r1 * OUT_D],
                        ot[:].rearrange("p a b -> p (a b)"))

    nc.compile()
    return nc


def _get_prog(w3j_flat):
    key = (W_DT, NT)
    if key not in _PROG:
        _PROG[key] = _build_program(w3j_flat)
    return _PROG[key]


# ---------------- entry point ----------------
def kernel(x_in, weights, bias_weights, w3j_flat, _trace=False):
    import ml_dtypes
    from concourse.bass_utils import run_bass_kernel_spmd

    x_in = np.asarray(x_in, dtype=np.float32)
    weights = np.asarray(weights, dtype=np.float32)
    bias_weights = np.asarray(bias_weights, dtype=np.float32)
    w3j_flat = np.asarray(w3j_flat, dtype=np.float32)

    nc = _get_prog(w3j_flat)
    np_dt = np.float32 if W_DT == "f32" else ml_dtypes.bfloat16

    rows = NT * TILE  # rows actually computed per core
    in_maps = []
    for c in range(NCORES):
        sl = slice(c * BS, c * BS + BS)
        wre, xb, biasp = _prep_core(x_in[sl], weights[sl], bias_weights[sl],
                                    w3j_flat, np_dt)
        in_maps.append(dict(wre=wre[:NT], xb=xb[:NT], biasp=biasp[:NT]))

    res = run_bass_kernel_spmd(nc, in_maps, core_ids=list(range(NCORES)),
                               trace=_trace)
    outs = [r["out"].reshape(rows, OUT_D, OUT_D) for r in res.results]
    if NT == BS // TILE:
        full = np.concatenate(outs, axis=0)
    else:  # debug partial build
        full = np.zeros((BATCH, OUT_D, OUT_D), np.float32)
        for c in range(NCORES):
            full[c * BS:c * BS + rows] = outs[c]
    kernel.last_exec_ns = res.exec_time_ns
    return full


kernel.last_exec_ns = None
